# revision 26
# baseline (speedup 1.0000x reference)
"""Bahdanau-attention scoring kernel for 8 TRN2 NeuronCores (fp8 DoubleRow).

Reference computation (S=2048, B=32, H=1024):
    cat    = concat([broadcast(hidden), enc], axis=2)          # [S,B,2H]
    alphas = tanh(einsum('sbk,hk->sbh', cat, W_attn) + b_attn) # [S,B,H]
    scores = einsum('sbh,h->sb', alphas, v)                    # [S,B]
    out    = softmax(scores.T, axis=1)[:, None, :]             # [B,1,S]

Because hidden broadcasts over S, the concat-matmul splits into
    z[s,b,:] = W2ᵀ enc[s,b,:] + hp[b,:],   hp[b,:] = W1ᵀ hidden[b] + b_attn.

The big matmul (2048·4·1024·1024 MACs per core) runs in fp8 e4m3 with
perf_mode=DoubleRow (2 fp8 MACs per PE cell per cycle).  Host pre-scales
enc×8 and W2×64 to keep values clear of the e4m3 subnormal range; the
1/512 descale rides on the DVE bias-add.  Offline sim with the exact
harness metric: 1.37e-2 mean rel err vs the 2e-2 gate (bf16: 1.5e-3).

Layout: h' (output feature) on partitions, s on the free dim, so that
  - the per-(b,h') bias hp enters as the [P,1] per-partition operand of a
    DVE scalar_tensor_tensor, broadcast along s (zq = z/512 + hp),
  - tanh is a plain ACT op,
  - the v-contraction (over h' = partitions) is a K=128/M=1 PE matmul.
    The 8 h'-tiles map to PE column groups 0/32/64/96 (explicit
    tile_position col-tiling) so 4 of these run concurrently in the
    array; the 4 partial rows are summed on the (otherwise idle) DVE.

Schedule notes:
  - v-matmuls are emitted in quads through a pending-FIFO that trails
    the fp8 stream by ~1 s-half, so the PE never waits on DVE/ACT.
  - hp's 8 matmul groups interleave with the first s-chunk's fp8 groups
    and w1 arrives in h'-major 256KB chunks, so the prologue is DMA-rate
    limited instead of serialized behind the full 2MB w1 load.
  - softmax exp runs per 512-row as scores complete; only the tiny
    total/reciprocal/scale work trails the last matmul.

Sharding: data-parallel over batch.  Core c handles batches 4c..4c+3.
"""

import sys

for _p in ("/opt/trn_rl_repo", "/root/.axon_site/_ro/trn_rl_repo"):
    if _p not in sys.path:
        sys.path.insert(0, _p)

import numpy as np
import ml_dtypes

import concourse.bass as bass  # noqa: F401  (bass must import before tile)
import concourse.mybir as mybir
import concourse.tile as tile
from concourse import bacc
from concourse.bass_utils import run_bass_kernel_spmd

S, B, H = 2048, 32, 1024
NCORES = 8
BL = B // NCORES          # batches per core (4)
P = 128                   # SBUF partitions
KT2 = H // P              # k-subtiles of 128 (8)
NKT = KT2 // 2            # DoubleRow k-pairs per z tile (4)
HT = H // P               # h'-tiles (8)
SC = 1024                 # s-chunk per enc DMA
NSC = S // SC             # 2
NROW = S // 512           # score rows of 512 per batch (4)

E_SCALE, W_SCALE = 8.0, 64.0
DESCALE = 1.0 / (E_SCALE * W_SCALE)

F8 = mybir.dt.float8e4
BF16 = mybir.dt.bfloat16
F32 = mybir.dt.float32
AFT = mybir.ActivationFunctionType
MUL = mybir.AluOpType.mult
ADD = mybir.AluOpType.add
DR = mybir.MatmulPerfMode.DoubleRow

SKEW = 3                  # pending-FIFO depth (quads/fins trailing the fp8 stream)

_nc_cache = None


def build():
    nc = bacc.Bacc()
    enc = nc.declare_dram_parameter("enc", [BL, H, S], F8, isOutput=False)
    w2 = nc.declare_dram_parameter("w2", [H, H], F8, isOutput=False)
    w1h = nc.declare_dram_parameter("w1h", [HT, P, H], BF16, isOutput=False)
    hid = nc.declare_dram_parameter("hid", [P, KT2 * BL], BF16, isOutput=False)
    ba = nc.declare_dram_parameter("ba", [1, H], BF16, isOutput=False)
    vv = nc.declare_dram_parameter("v", [P, HT], BF16, isOutput=False)
    out = nc.declare_dram_parameter("out", [BL, S], F32, isOutput=True)

    with tile.TileContext(nc) as tc:
        with (
            tc.tile_pool(name="const", bufs=1) as cpool,
            tc.tile_pool(name="encp", bufs=12) as encp,
            tc.tile_pool(name="zqp", bufs=4) as zqp,
            tc.tile_pool(name="alqp", bufs=12) as alqp,
            tc.tile_pool(name="smallp", bufs=2) as smallp,
            tc.tile_pool(name="zps", bufs=3, space="PSUM") as zps,
            tc.tile_pool(name="sps", bufs=2, space="PSUM") as sps,
        ):
            # --- resident constants, one tile per DMA so consumers wait
            # only on their own 256KB chunk, staggered over three queues
            # to co-time w2/et (feeding the fp8 stream) with w1 (feeding
            # hp, which gates the DVE bias-add) ---
            hid_sb = cpool.tile([P, KT2, BL], BF16)
            nc.scalar.dma_start(hid_sb[:], hid[:])   # host pre-arranged
            ba_sb = cpool.tile([1, H], BF16)
            nc.scalar.dma_start(ba_sb[:], ba[:])
            v_sb = cpool.tile([P, HT], BF16)
            nc.scalar.dma_start(v_sb[:], vv[:])
            w1t = [cpool.tile([P, H], BF16, tag=f"w1_{ht}", name=f"w1_{ht}")
                   for ht in range(HT)]
            w2p = [cpool.tile([P, 2, H], F8, tag=f"w2_{kt}", name=f"w2_{kt}")
                   for kt in range(NKT)]
            for ht in range(4):
                nc.scalar.dma_start(w1t[ht][:], w1h[ht])
            for kt in range(NKT):
                nc.gpsimd.dma_start(
                    w2p[kt][:], w2[2 * kt * P:(2 * kt + 2) * P, :].rearrange(
                        "(t p) h -> p t h", p=P))
            nc.gpsimd.dma_start(w1t[4][:], w1h[4])
            nc.gpsimd.dma_start(w1t[5][:], w1h[5])
            ones1 = cpool.tile([1, BL], BF16)
            nc.vector.memset(ones1[:], 1.0)
            hp_t = cpool.tile([P, HT, BL], F32)      # hp, h'-major

            def emit_hp(ht):
                # hp[b,:] = W1ᵀ hidden[b] + b_attn for one h'-tile
                hp_ps = sps.tile([P, BL], F32, tag="srow", name=f"hp{ht}")
                for kt in range(KT2):
                    nc.tensor.matmul(
                        hp_ps[:], w1t[ht][:, kt * P:(kt + 1) * P],
                        hid_sb[:, kt, :], start=(kt == 0), stop=False)
                # + b_attn as a K=1 rank-1 update (ba ⊗ ones)
                nc.tensor.matmul(
                    hp_ps[:], ba_sb[:, ht * P:(ht + 1) * P], ones1[:],
                    start=False, stop=True)
                nc.vector.tensor_copy(hp_t[:, ht, :], hp_ps[:])

            # --- main loop ---
            pending = []

            def drain(n):
                while len(pending) > n:
                    pending.pop(0)()

            for b in range(BL):
                scb = smallp.tile([1, S], F32, tag="sb", name=f"scb{b}")
                for sc in range(NSC):
                    chunk = b * NSC + sc
                    q = nc.gpsimd if chunk in (2, 4, 6) else nc.sync
                    etp = [encp.tile([P, 2, SC], F8, tag="enc",
                                     name=f"et{chunk}_{kt}")
                           for kt in range(NKT)]
                    for kt in range(NKT):   # kt-pair tiles: MMs can start
                        q.dma_start(        # before the full chunk lands
                            etp[kt][:],
                            enc[b, 2 * kt * P:(2 * kt + 2) * P,
                                sc * SC:(sc + 1) * SC].rearrange(
                                    "(t p) s -> p t s", p=P))
                    if chunk == 0:
                        nc.sync.dma_start(w1t[6][:], w1h[6])
                        nc.sync.dma_start(w1t[7][:], w1h[7])
                    for half in range(2):
                        row = sc * 2 + half
                        first = b == 0 and sc == 0 and half == 0
                        score_ps = sps.tile([P, 512], F32, tag="srow",
                                            name=f"srow{b}_{row}")
                        quad = []
                        zw = []
                        if first:
                            # ramp: consume kt-pairs as they land — kt-outer
                            # across the first 3 pair-groups (6 PSUM banks),
                            # with hp's matmuls riding behind
                            zw = [zps.tile([P, 2, 512], F32, tag="z",
                                           name=f"zw{i}") for i in range(3)]
                            for kt in range(NKT):
                                for hq in range(3):
                                    for ht2 in range(2):
                                        ht = hq * 2 + ht2
                                        nc.tensor.matmul(
                                            zw[hq][:, ht2, :],
                                            w2p[kt][:, :, ht * P:(ht + 1) * P],
                                            etp[kt][:, :, 0:512],
                                            start=(kt == 0),
                                            stop=(kt == NKT - 1),
                                            perf_mode=DR)
                            for j in range(6):
                                emit_hp(j)
                        for htp in range(HT // 2):
                            if first and htp < 3:
                                z_ps = zw[htp]
                            else:
                                z_ps = zps.tile([P, 2, 512], F32, tag="z")
                                for ht2 in range(2):
                                    ht = htp * 2 + ht2
                                    for kt in range(NKT):
                                        nc.tensor.matmul(
                                            z_ps[:, ht2, :],
                                            w2p[kt][:, :, ht * P:(ht + 1) * P],
                                            etp[kt][:, :,
                                                    half * 512:(half + 1) * 512],
                                            start=(kt == 0),
                                            stop=(kt == NKT - 1),
                                            perf_mode=DR)
                            if first and htp == 3:
                                emit_hp(6)
                                emit_hp(7)
                            # z-eviction on DVE (the engine with slack):
                            # zq = z/512 + hp, then tanh on ACT
                            zq = zqp.tile([P, 2, 512], BF16, tag="zq")
                            nc.vector.scalar_tensor_tensor(
                                zq[:], z_ps[:], DESCALE,
                                hp_t[:, htp * 2:htp * 2 + 2,
                                     b:b + 1].broadcast_to((P, 2, 512)),
                                op0=MUL, op1=ADD)
                            alq = alqp.tile([P, 2, 512], BF16, tag="alq")
                            nc.scalar.activation(alq[:], zq[:], AFT.Tanh)
                            quad.append((htp, alq))
                            if htp % 2 == 1:
                                # v-contraction: 4 concurrent M=1 matmuls on
                                # PE column groups 0/32/64/96 — slot j owns
                                # s-quarter j and accumulates all 8 h'-tiles,
                                # so no cross-slot sum is needed
                                def vmms(quad=tuple(quad), score_ps=score_ps):
                                    for hq, a in quad:
                                        for ht2 in range(2):
                                            ht = hq * 2 + ht2
                                            for j in range(4):
                                                nc.tensor.matmul(
                                                    score_ps[32 * j:32 * j + 1,
                                                             0:P],
                                                    v_sb[:, ht:ht + 1],
                                                    a[:, ht2, j * P:(j + 1) * P],
                                                    start=(ht == 0),
                                                    stop=(ht == HT - 1),
                                                    tile_position=(0, 32 * j))
                                pending.append(vmms)
                                quad = []
                            drain(1 if chunk == BL * NSC - 1 else SKEW)
                        # gather the 4 column-group slots into the batch's
                        # score row on the (idle) DVE
                        def fin(b=b, row=row, score_ps=score_ps, scb=scb):
                            for j in range(4):
                                nc.vector.tensor_copy(
                                    scb[:, row * 512 + j * P:
                                        row * 512 + (j + 1) * P],
                                    score_ps[32 * j:32 * j + 1, 0:P])
                        pending.append(fin)

                # softmax row b (no max-sub: |scores| <= sum|v| ~ 26)
                def softmax(b=b, scb=scb):
                    ex = smallp.tile([1, S], F32, tag="ex")
                    tot = smallp.tile([1, 1], F32, tag="tot", bufs=2)
                    nc.scalar.activation(ex[:], scb[:], AFT.Exp,
                                         accum_out=tot[:])
                    rec = smallp.tile([1, 1], F32, tag="rec", bufs=2)
                    nc.vector.reciprocal(rec[:], tot[:])
                    osb = smallp.tile([1, S], F32, tag="osb")
                    nc.vector.tensor_scalar_mul(osb[:], ex[:], rec[:, 0:1])
                    nc.scalar.dma_start(out[b:b + 1, :], osb[:])
                pending.append(softmax)
            drain(0)
    nc.compile()
    return nc


def _get_nc():
    global _nc_cache
    if _nc_cache is None:
        _nc_cache = build()
    return _nc_cache


def kernel(hidden, encoder_outputs, W_attn, b_attn, v, _trace=False):
    f8 = ml_dtypes.float8_e4m3
    bf16 = ml_dtypes.bfloat16
    hidden = np.asarray(hidden, dtype=np.float32)
    encoder_outputs = np.asarray(encoder_outputs, dtype=np.float32)
    W_attn = np.asarray(W_attn, dtype=np.float32)
    b_attn = np.asarray(b_attn, dtype=np.float32)
    v = np.asarray(v, dtype=np.float32)

    w1t = W_attn[:, :H].T                                        # [k, h']
    # h'-major 256KB chunks: w1h[ht, p, kt*128+m] = W1[kt*128+p, ht*128+m]
    w1h = np.ascontiguousarray(
        w1t.reshape(KT2, P, HT, P).transpose(2, 1, 0, 3).reshape(HT, P, H)
    ).astype(bf16)
    w2 = np.ascontiguousarray(W_attn[:, H:].T * W_SCALE).astype(f8)
    hid_t = hidden[0].T                                          # [H, B]
    ba = b_attn.reshape(1, H).astype(bf16)
    vv = np.ascontiguousarray(v.reshape(HT, P).T).astype(bf16)   # [P, HT]
    # [B, H, S] b-major, s-contiguous, pre-scaled fp8
    enc_t = (encoder_outputs.transpose(1, 2, 0) * E_SCALE).astype(f8)

    in_maps = []
    for c in range(NCORES):
        bsl = slice(c * BL, (c + 1) * BL)
        in_maps.append({
            "enc": np.ascontiguousarray(enc_t[bsl]),
            "w2": w2,
            "w1h": w1h,
            "hid": np.ascontiguousarray(
                hid_t[:, bsl].reshape(KT2, P, BL).transpose(1, 0, 2)
                .reshape(P, KT2 * BL)).astype(bf16),
            "ba": ba,
            "v": vv,
        })

    nc = _get_nc()
    res = run_bass_kernel_spmd(
        nc, in_maps, core_ids=list(range(NCORES)), trace=_trace,
    )
    parts = [res.results[c]["out"] for c in range(NCORES)]      # [BL, S] each
    full = np.concatenate(parts, axis=0)
    out = full[:, None, :].astype(np.float32)                   # [B, 1, S]
    if _trace:
        return out, res
    return out
